# revision 1
# baseline (speedup 1.0000x reference)
"""Multi-head attention (B=8, N=1024, C=768, H=12, D=64) on 8 TRN2 NeuronCores.

Strategy: pure data-parallel over batch (B == n_cores == 8), no collectives.
Each core computes full 12-head attention for one batch element, in a fully
transposed layout (channels on SBUF partitions) so no on-device transposes are
needed:

  per core:  xT=[C,N] -> QT,KT=[C,N], V=[N,C] (+ ones col)
             per (head, nk-tile): S^T = K_h Q_h^T  into PSUM [128 nk, 1024 nq]
             S^T += 8*bias^T (DVE);  P^T = exp(0.125*S^T) (ACT -> bf16)
             PV:  [V_h | 1]^T @ P^T -> rows 0:64 = out_h^T (unnorm), row 64 = sum
             softmax sums collected, batch-reciprocal in two halves (overlapped
             with attention), broadcast once per half, normalize on GpSimd,
             out^T = Wp @ attnT + bp -> DMA out, host transposes back.

K/Q projection tiles are interleaved at head boundaries so the TensorEngine
stays dense (HAM un-throttled). Matmuls in bf16 with f32 PSUM accumulation.
"""

import os
import sys
import numpy as np

for _p in ("/opt/trn_rl_repo", "/root/.axon_site/_ro/trn_rl_repo"):
    if os.path.isdir(_p) and _p not in sys.path:
        sys.path.append(_p)

import ml_dtypes

BF16 = ml_dtypes.bfloat16

B, N, C = 8, 1024, 768
H, D = 12, 64
CT = C // 128        # 6 channel tiles
NT = N // 128        # 8 key tiles
F = 512
HA = 8               # heads in normalization batch A (rest in batch B)

_cache = {}


def _build():
    import concourse.bass as bass
    import concourse.tile as tile
    from concourse import bacc, mybir

    f32 = mybir.dt.float32
    bf16 = mybir.dt.bfloat16
    AF = mybir.ActivationFunctionType
    ALU = mybir.AluOpType

    nc = bacc.Bacc("TRN2", target_bir_lowering=False)

    xT_d = nc.dram_tensor("xT", [C, N], bf16, kind="ExternalInput")
    wqT_d = nc.dram_tensor("wqT", [C, C], bf16, kind="ExternalInput")
    wkT_d = nc.dram_tensor("wkT", [C, C], bf16, kind="ExternalInput")
    wvT_d = nc.dram_tensor("wvT", [C, C], bf16, kind="ExternalInput")
    wpT_d = nc.dram_tensor("wpT", [C, C], bf16, kind="ExternalInput")
    bpT_d = nc.dram_tensor("bpT", [128, CT], f32, kind="ExternalInput")
    biasT8_d = nc.dram_tensor("biasT8", [H, N, N], bf16, kind="ExternalInput")
    outT_d = nc.dram_tensor("outT", [C, N], f32, kind="ExternalOutput")
    # softmax-sum scratch: batch A = heads 0..7, batch B = heads 8..11
    sA_scr = nc.dram_tensor("sA_scr", [HA * N], bf16)
    sB_scr = nc.dram_tensor("sB_scr", [(H - HA) * N], bf16)
    rA_scr = nc.dram_tensor("rA_scr", [1, HA * N], bf16)
    rB_scr = nc.dram_tensor("rB_scr", [1, (H - HA) * N], bf16)

    with tile.TileContext(nc) as tc:
        with tc.tile_pool(name="persist", bufs=1) as pers:
            xTb = pers.tile([128, CT, N], bf16, tag="xT")
            wqb = pers.tile([128, CT, C], bf16, tag="wq")
            wkb = pers.tile([128, CT, C], bf16, tag="wk")
            wvb = pers.tile([128, CT, C], bf16, tag="wv")
            wpb = pers.tile([128, CT, C], bf16, tag="wp")
            bpb = pers.tile([128, CT], f32, tag="bp")
            # row 64 collects softmax sums (same start partition as pv row 64)
            s_stage = pers.tile([65, H * N], bf16, tag="s_stage")
            rba = pers.tile([128, H * N], bf16, tag="rba")
            qtb = pers.tile([128, CT, N], bf16, tag="qt")
            ktb = pers.tile([128, CT, N], bf16, tag="kt")
            vb = pers.tile([128, NT, H, D + 1], bf16, tag="v")
            atb = pers.tile([128, CT, N], bf16, tag="at")

            for q0 in range(0, N, 256):
                nc.sync.dma_start(
                    xTb[:, :, q0:q0 + 256],
                    xT_d[:, q0:q0 + 256].rearrange(
                        "(ci p) n -> p ci n", p=128))
            nc.scalar.dma_start(
                wvb[:, :, 0:512],
                wvT_d[:, 0:512].rearrange("(ci p) o -> p ci o", p=128))
            nc.scalar.dma_start(
                wvb[:, :, 512:C],
                wvT_d[:, 512:C].rearrange("(ci p) o -> p ci o", p=128))
            nc.sync.dma_start(
                wkb, wkT_d[:].rearrange("(ci p) o -> p ci o", p=128))
            nc.scalar.dma_start(
                wqb, wqT_d[:].rearrange("(ci p) o -> p ci o", p=128))
            nc.sync.dma_start(
                wpb, wpT_d[:].rearrange("(ci p) o -> p ci o", p=128))
            nc.scalar.dma_start(bpb, bpT_d[:])

            nc.vector.memset(vb[:, :, :, D:D + 1], 1.0)

            with tc.tile_pool(name="ups", bufs=6, space="PSUM") as pU, \
                 tc.tile_pool(name="pvps", bufs=2, space="PSUM") as pPV, \
                 tc.tile_pool(name="biasb", bufs=2) as biasp, \
                 tc.tile_pool(name="vstagb", bufs=4) as vstagp, \
                 tc.tile_pool(name="nrmb", bufs=1) as nrm, \
                 tc.tile_pool(name="ptb", bufs=12) as ptp:

                def v_proj(block, nts):
                    f0, fw, h0 = (0, 512, 0) if block == 0 else (512, 256, 8)
                    for nt in nts:
                        ps = pU.tile([128, F], f32, tag="ps")
                        for ci in range(CT):
                            nc.tensor.matmul(
                                ps[:, :fw],
                                lhsT=xTb[:, ci, nt * 128:(nt + 1) * 128],
                                rhs=wvb[:, ci, f0:f0 + fw],
                                start=(ci == 0),
                                stop=(ci == CT - 1),
                            )
                        nc.vector.tensor_copy(
                            vb[:, nt, h0:h0 + fw // D, 0:D],
                            ps[:, :fw].rearrange("p (h d) -> p h d", d=D),
                        )

                def kq_sub(which, cot, nb):
                    wb, dst = (wkb, ktb) if which == "k" else (wqb, qtb)
                    ps = pU.tile([128, F], f32, tag="ps")
                    for ci in range(CT):
                        nc.tensor.matmul(
                            ps[:, :F],
                            lhsT=wb[:, ci, cot * 128:(cot + 1) * 128],
                            rhs=xTb[:, ci, nb * F:(nb + 1) * F],
                            start=(ci == 0),
                            stop=(ci == CT - 1),
                        )
                    nc.vector.tensor_copy(
                        dst[:, cot, nb * F:(nb + 1) * F], ps[:, :F])

                def kq_ct(cot):
                    for which in ("k", "q"):
                        for nb in range(2):
                            kq_sub(which, cot, nb)

                def attn(h):
                    ct, po = h // 2, 64 * (h % 2)
                    bt = biasp.tile([128, NT, N], bf16, tag="bt")
                    bsrc = biasT8_d[h].rearrange("(j p) q -> p j q", p=128)
                    nc.sync.dma_start(bt[:, 0:NT // 2, :], bsrc[:, 0:NT // 2, :])
                    nc.sync.dma_start(bt[:, NT // 2:, :], bsrc[:, NT // 2:, :])
                    for nb in range(2):
                        pv = pPV.tile([D + 1, F], f32, tag="pv")
                        qsl = slice(nb * F, (nb + 1) * F)
                        for j in range(NT):
                            ps = pU.tile([128, F], f32, tag="ps")
                            nc.tensor.matmul(
                                ps,
                                lhsT=ktb[po:po + 64, ct, j * 128:(j + 1) * 128],
                                rhs=qtb[po:po + 64, ct, qsl],
                                start=True,
                                stop=True,
                            )
                            nc.vector.tensor_tensor(
                                ps, ps, bt[:, j, qsl], ALU.add)
                            pt = ptp.tile([128, F], bf16, tag="pt")
                            nc.scalar.activation(pt, ps, AF.Exp, scale=0.125)
                            nc.tensor.matmul(
                                pv,
                                lhsT=vb[:, j, h, :],
                                rhs=pt,
                                start=(j == 0),
                                stop=(j == NT - 1),
                            )
                        # evacuate: hidden under the other nq-half's loop
                        dst = atb[po:po + 64, ct, qsl]
                        if po == 0:
                            nc.vector.tensor_copy(dst, pv[0:D, :])
                        else:
                            vstag = vstagp.tile([D, F], bf16, tag="vstag")
                            nc.vector.tensor_copy(vstag, pv[0:D, :])
                            nc.gpsimd.dma_start(dst, vstag)
                        nc.scalar.copy(
                            s_stage[D:D + 1, h * N + nb * F:
                                    h * N + (nb + 1) * F],
                            pv[D:D + 1, :])

                def norm_batch(batch):
                    """Batched reciprocal of softmax sums for a head range."""
                    h0, nh = (0, HA) if batch == 0 else (HA, H - HA)
                    s_scr = sA_scr if batch == 0 else sB_scr
                    r_scr = rA_scr if batch == 0 else rB_scr
                    cols = nh * N // 128
                    nc.scalar.dma_start(
                        s_scr[:], s_stage[D:D + 1, h0 * N:(h0 + nh) * N])
                    sb = nrm.tile([128, H * N // 128], bf16, tag="sb")
                    nc.scalar.dma_start(
                        sb[:, :cols],
                        s_scr[:].rearrange("(p f) -> p f", p=128))
                    rc32 = nrm.tile([128, H * N // 128], f32, tag="rc32")
                    nc.vector.reciprocal(rc32[:, :cols], sb[:, :cols])
                    rcb = nrm.tile([128, H * N // 128], bf16, tag="rcb")
                    nc.vector.tensor_copy(rcb[:, :cols], rc32[:, :cols])
                    nc.scalar.dma_start(
                        r_scr[0, :].rearrange("(p f) -> p f", p=128),
                        rcb[:, :cols])
                    nc.sync.dma_start(
                        rba[:, h0 * N:(h0 + nh) * N],
                        r_scr[:].to_broadcast([128, nh * N]))

                def norm_mul(h):
                    ct, po = h // 2, 64 * (h % 2)
                    sl = atb[po:po + 64, ct, :]
                    nc.gpsimd.tensor_tensor(
                        sl, sl, rba[po:po + 64, h * N:(h + 1) * N], ALU.mult)

                # emission order: kq_ct(ct) fully before attn(2ct); filler
                # spread across head boundaries to keep PE dense
                v_proj(0, range(NT))
                kq_ct(0)
                attn(0)
                kq_sub("k", 1, 0); kq_sub("k", 1, 1)
                attn(1)
                kq_sub("q", 1, 0); kq_sub("q", 1, 1)
                attn(2)
                kq_sub("k", 2, 0); kq_sub("k", 2, 1)
                attn(3)
                kq_sub("q", 2, 0); kq_sub("q", 2, 1)
                attn(4)
                kq_sub("k", 3, 0); kq_sub("k", 3, 1)
                attn(5)
                kq_sub("q", 3, 0); kq_sub("q", 3, 1)
                v_proj(1, range(0, 4))
                attn(6)
                kq_sub("k", 4, 0); kq_sub("k", 4, 1)
                v_proj(1, range(4, NT))
                attn(7)
                kq_sub("q", 4, 0); kq_sub("q", 4, 1)
                norm_batch(0)          # heads 0..7: overlapped with attn 8,9
                attn(8)
                kq_sub("k", 5, 0); kq_sub("k", 5, 1)
                norm_mul(0); norm_mul(1); norm_mul(2); norm_mul(3)
                attn(9)
                kq_sub("q", 5, 0); kq_sub("q", 5, 1)
                norm_mul(4); norm_mul(5); norm_mul(6); norm_mul(7)
                attn(10)
                attn(11)
                norm_batch(1)          # heads 8..11
                norm_mul(8); norm_mul(9); norm_mul(10); norm_mul(11)

            # ---- output projection ------------------------------------------
            with tc.tile_pool(name="ops", bufs=8, space="PSUM") as pC, \
                 tc.tile_pool(name="otb", bufs=3) as otp:
                for cot in range(CT):
                    for nb in range(2):
                        ps = pC.tile([128, F], f32, tag="o")
                        for ci in range(CT):
                            nc.tensor.matmul(
                                ps,
                                lhsT=wpb[:, ci, cot * 128:(cot + 1) * 128],
                                rhs=atb[:, ci, nb * F:(nb + 1) * F],
                                start=(ci == 0),
                                stop=(ci == CT - 1),
                            )
                        ot = otp.tile([128, F], f32, tag="ot")
                        nc.scalar.activation(
                            ot, ps, AF.Identity, bias=bpb[:, cot:cot + 1])
                        nc.scalar.dma_start(
                            outT_d[cot * 128:(cot + 1) * 128,
                                   nb * F:(nb + 1) * F],
                            ot,
                        )

    nc.compile()
    return nc


def _get_nc():
    if "nc" not in _cache:
        _cache["nc"] = _build()
    return _cache["nc"]


def prep_in_maps(x, attn_bias, Wq, Wk, Wv, Wp, bp):
    """Host-side sharding + layout prep (transposes/casts only)."""
    wqT = np.ascontiguousarray(Wq.T).astype(BF16)
    wkT = np.ascontiguousarray(Wk.T).astype(BF16)
    wvT = np.ascontiguousarray(Wv.T).astype(BF16)
    wpT = np.ascontiguousarray(Wp.T).astype(BF16)
    bpT = np.ascontiguousarray(bp.astype(np.float32).reshape(CT, 128).T)
    biasT8 = np.ascontiguousarray(
        (attn_bias[0].astype(np.float32) * 8.0).transpose(0, 2, 1)
    ).astype(BF16)
    in_maps = []
    for b in range(B):
        in_maps.append({
            "xT": np.ascontiguousarray(x[b].T).astype(BF16),
            "wqT": wqT, "wkT": wkT, "wvT": wvT, "wpT": wpT,
            "bpT": bpT, "biasT8": biasT8,
        })
    return in_maps


def run(in_maps, trace=False, **kw):
    from concourse.bass_utils import run_bass_kernel_spmd

    nc = _get_nc()
    return run_bass_kernel_spmd(
        nc, in_maps, core_ids=list(range(B)), trace=trace, **kw
    )


def kernel(x, attn_bias, Wq, Wk, Wv, Wp, bp):
    res = run(prep_in_maps(x, attn_bias, Wq, Wk, Wv, Wp, bp))
    out = np.stack(
        [res.results[b]["outT"].T for b in range(B)]
    ).astype(np.float32)
    return out



# revision 4
# speedup vs baseline: 1.1745x; 1.1745x over previous
"""Multi-head attention (B=8, N=1024, C=768, H=12, D=64) on 8 TRN2 NeuronCores.

Strategy: pure data-parallel over batch (B == n_cores == 8), no collectives.
Each core computes full 12-head attention for one batch element in a fully
transposed layout (channels on SBUF partitions).

v2 design (vs. the nb-serial baseline):
  - Heads are processed in PAIRS (2i, 2i+1).  The even head's K/Q live on
    SBUF partitions 0:64, the odd head's on 64:128, so the two QK^T matmuls
    (contraction 64 each) run CONCURRENTLY in the PE array via row tiling
    (tile_position (0,0) / (64,0)) writing different PSUM banks.
  - S-pair tiles are [128, 1024] f32 (2 PSUM banks): cols 0:512 even head,
    512:1024 odd head, for one (key-tile j, query-half nb).  One FD=1024
    ACT exp per tile halves the per-instruction overhead.
  - The additive attn bias is applied as exp(S/8)*exp(bias): exp(bias) is
    precomputed on host, loaded bf16, multiplied on DVE at 2x rate in SBUF
    (the f32-PSUM add of the baseline ran at 1x and cost 131us).
  - PV keeps the ones-column trick (out rows 0:64 = unnormalized out^T,
    row 64 = softmax sum).  All four (head, nb) chains evacuate via one
    [65,512] DVE copy to SBUF, then gpsimd DMAs split rows 0:64 -> atb
    (partition-shifted for odd heads) and row 64 -> s_stage.
  - Normalization in 3 batches (heads 0-7, 8-9, 10-11) overlapped with
    attention; the last two heads' normalize-multiply runs on DVE to
    shorten the tail.  Whole norm DMA chain rides the sync queue (FIFO).
  - K/Q projection tiles for pair i+1 are interleaved into pair i's
    attention to keep the PE dense (HAM un-throttled).
"""

import os
import sys
import numpy as np

for _p in ("/opt/trn_rl_repo", "/root/.axon_site/_ro/trn_rl_repo"):
    if os.path.isdir(_p) and _p not in sys.path:
        sys.path.append(_p)

import ml_dtypes

BF16 = ml_dtypes.bfloat16

B, N, C = 8, 1024, 768
H, D = 12, 64
CT = C // 128         # 6 channel tiles
NT = N // 128         # 8 key tiles
F = 512
NP = H // 2           # 6 head pairs
HA = 8                # heads in normalization batch A (then 8-9, 10-11)

_cache = {}


def _build():
    import concourse.bass as bass
    import concourse.tile as tile
    from concourse import bacc, mybir

    f32 = mybir.dt.float32
    bf16 = mybir.dt.bfloat16
    AF = mybir.ActivationFunctionType
    ALU = mybir.AluOpType

    nc = bacc.Bacc("TRN2", target_bir_lowering=False)

    xT_d = nc.dram_tensor("xT", [C, N], bf16, kind="ExternalInput")
    wqT_d = nc.dram_tensor("wqT", [C, C], bf16, kind="ExternalInput")
    wkT_d = nc.dram_tensor("wkT", [C, C], bf16, kind="ExternalInput")
    wvT_d = nc.dram_tensor("wvT", [C, C], bf16, kind="ExternalInput")
    wpT_d = nc.dram_tensor("wpT", [C, C], bf16, kind="ExternalInput")
    bpT_d = nc.dram_tensor("bpT", [128, CT], f32, kind="ExternalInput")
    # exp(attn_bias) packed per (pair, key-tile j, query-half nb):
    # [...,0:512] = even head, [...,512:1024] = odd head
    eb_d = nc.dram_tensor("ebPk", [NP, NT, 2, 128, 2 * F], bf16,
                          kind="ExternalInput")
    outT_d = nc.dram_tensor("outT", [C, N], f32, kind="ExternalOutput")
    # softmax-sum scratch per normalization batch
    sA_scr = nc.dram_tensor("sA_scr", [HA * N], bf16)
    sB1_scr = nc.dram_tensor("sB1_scr", [2 * N], bf16)
    sB2_scr = nc.dram_tensor("sB2_scr", [2 * N], bf16)
    rA_scr = nc.dram_tensor("rA_scr", [1, HA * N], bf16)
    rB1_scr = nc.dram_tensor("rB1_scr", [1, 2 * N], bf16)
    rB2_scr = nc.dram_tensor("rB2_scr", [1, 2 * N], bf16)

    with tile.TileContext(nc) as tc:
        with tc.tile_pool(name="persist", bufs=1) as pers:
            xTb = pers.tile([128, CT, N], bf16, tag="xT")
            wqb = pers.tile([128, CT, C], bf16, tag="wq")
            wkb = pers.tile([128, CT, C], bf16, tag="wk")
            wvb = pers.tile([128, CT, C], bf16, tag="wv")
            wpb = pers.tile([128, CT, C], bf16, tag="wp")
            bpb = pers.tile([128, CT], f32, tag="bp")
            # row 64 collects softmax sums (same partition as pv row 64)
            s_stage = pers.tile([65, H * N], bf16, tag="s_stage")
            rba = pers.tile([128, H * N], bf16, tag="rba")
            qtb = pers.tile([128, CT, N], bf16, tag="qt")
            ktb = pers.tile([128, CT, N], bf16, tag="kt")
            vb = pers.tile([128, NT, H, D + 1], bf16, tag="v")
            atb = pers.tile([128, CT, N], bf16, tag="at")

            for q0 in range(0, N, 256):
                nc.sync.dma_start(
                    xTb[:, :, q0:q0 + 256],
                    xT_d[:, q0:q0 + 256].rearrange(
                        "(ci p) n -> p ci n", p=128))
            nc.scalar.dma_start(
                wvb[:, :, 0:512],
                wvT_d[:, 0:512].rearrange("(ci p) o -> p ci o", p=128))
            nc.scalar.dma_start(
                wvb[:, :, 512:C],
                wvT_d[:, 512:C].rearrange("(ci p) o -> p ci o", p=128))
            nc.sync.dma_start(
                wkb, wkT_d[:].rearrange("(ci p) o -> p ci o", p=128))
            nc.scalar.dma_start(
                wqb, wqT_d[:].rearrange("(ci p) o -> p ci o", p=128))
            nc.sync.dma_start(
                wpb, wpT_d[:].rearrange("(ci p) o -> p ci o", p=128))
            nc.scalar.dma_start(bpb, bpT_d[:])

            nc.vector.memset(vb[:, :, :, D:D + 1], 1.0)

            with tc.tile_pool(name="ups", bufs=2, space="PSUM") as pU, \
                 tc.tile_pool(name="pvps", bufs=4, space="PSUM") as pPV, \
                 tc.tile_pool(name="ebb", bufs=4) as ebp, \
                 tc.tile_pool(name="vstagb", bufs=4) as vstagp, \
                 tc.tile_pool(name="nrmb", bufs=1) as nrm, \
                 tc.tile_pool(name="ptb", bufs=6) as ptp:

                eb_tiles = {}

                def eb_load(pr, j):
                    ebt = ebp.tile([128, 2, 2 * F], bf16, tag="eb")
                    nc.sync.dma_start(
                        ebt, eb_d[pr, j].rearrange("nb p q -> p nb q"))
                    eb_tiles[(pr, j)] = ebt

                def v_proj(block, nts):
                    f0, fw, h0 = (0, 512, 0) if block == 0 else (512, 256, 8)
                    for nt in nts:
                        ps = pU.tile([128, 2 * F], f32, tag="ps")
                        for ci in range(CT):
                            nc.tensor.matmul(
                                ps[:, :fw],
                                lhsT=xTb[:, ci, nt * 128:(nt + 1) * 128],
                                rhs=wvb[:, ci, f0:f0 + fw],
                                start=(ci == 0),
                                stop=(ci == CT - 1),
                            )
                        nc.vector.tensor_copy(
                            vb[:, nt, h0:h0 + fw // D, 0:D],
                            ps[:, :fw].rearrange("p (h d) -> p h d", d=D),
                        )

                def kq_sub(which, cot, nb):
                    wb, dst = (wkb, ktb) if which == "k" else (wqb, qtb)
                    ps = pU.tile([128, 2 * F], f32, tag="ps")
                    for ci in range(CT):
                        nc.tensor.matmul(
                            ps[:, :F],
                            lhsT=wb[:, ci, cot * 128:(cot + 1) * 128],
                            rhs=xTb[:, ci, nb * F:(nb + 1) * F],
                            start=(ci == 0),
                            stop=(ci == CT - 1),
                        )
                    nc.vector.tensor_copy(
                        dst[:, cot, nb * F:(nb + 1) * F], ps[:, :F])

                def kq_ct(cot):
                    for which in ("k", "q"):
                        for nb in range(2):
                            kq_sub(which, cot, nb)

                def attn_pair(pr, fillers):
                    """Attention for heads (2*pr, 2*pr+1).

                    fillers: dict j -> list of callables emitted after
                    that j-iteration (projection tiles, EB prefetch,
                    norm batches) to keep the PE dense.
                    """
                    pvs = [[pPV.tile([D + 1, F], f32, tag="pv",
                                     name=f"pv_{pr}_{hl}_{nb}")
                            for nb in range(2)] for hl in range(2)]
                    for j in range(NT):
                        ebt = eb_tiles.pop((pr, j))
                        ksl = slice(j * 128, (j + 1) * 128)
                        for nb in range(2):
                            qsl = slice(nb * F, (nb + 1) * F)
                            sp = pU.tile([128, 2 * F], f32, tag="ps",
                                         name=f"sp_{pr}_{j}_{nb}")
                            nc.tensor.matmul(
                                sp[:, 0:F],
                                lhsT=ktb[0:64, pr, ksl],
                                rhs=qtb[0:64, pr, qsl],
                                start=True, stop=True,
                            )
                            nc.tensor.matmul(
                                sp[:, F:2 * F],
                                lhsT=ktb[64:128, pr, ksl],
                                rhs=qtb[64:128, pr, qsl],
                                start=True, stop=True,
                            )
                            pt = ptp.tile([128, 2 * F], bf16, tag="pt",
                                          name=f"pt_{pr}_{j}_{nb}")
                            nc.scalar.activation(pt, sp, AF.Exp, scale=0.125)
                            nc.vector.tensor_tensor(
                                pt, pt, ebt[:, nb, :], ALU.mult)
                            for hl in range(2):
                                nc.tensor.matmul(
                                    pvs[hl][nb],
                                    lhsT=vb[:, j, 2 * pr + hl, :],
                                    rhs=pt[:, hl * F:(hl + 1) * F],
                                    start=(j == 0),
                                    stop=(j == NT - 1),
                                )
                        for fn in fillers.get(j, ()):
                            fn()
                    # evacuate the 4 chains
                    for hl in range(2):
                        h = 2 * pr + hl
                        for nb in range(2):
                            qsl = slice(nb * F, (nb + 1) * F)
                            vstag = vstagp.tile([D + 1, F], bf16, tag="vstag")
                            nc.vector.tensor_copy(vstag, pvs[hl][nb])
                            nc.gpsimd.dma_start(
                                atb[64 * hl:64 * hl + 64, pr, qsl],
                                vstag[0:D, :])
                            nc.gpsimd.dma_start(
                                s_stage[D:D + 1,
                                        h * N + nb * F:h * N + (nb + 1) * F],
                                vstag[D:D + 1, :])

                def norm_batch(batch):
                    """Batched reciprocal of softmax sums for a head range.

                    All DMAs ride the sync queue so the DRAM round trip
                    is FIFO-ordered.
                    """
                    h0, nh, s_scr, r_scr = [
                        (0, HA, sA_scr, rA_scr),
                        (HA, 2, sB1_scr, rB1_scr),
                        (HA + 2, 2, sB2_scr, rB2_scr),
                    ][batch]
                    cols = nh * N // 128
                    nc.sync.dma_start(
                        s_scr[:], s_stage[D:D + 1, h0 * N:(h0 + nh) * N])
                    sb = nrm.tile([128, HA * N // 128], bf16, tag="sb")
                    nc.sync.dma_start(
                        sb[:, :cols],
                        s_scr[:].rearrange("(p f) -> p f", p=128))
                    rc32 = nrm.tile([128, HA * N // 128], f32, tag="rc32")
                    nc.vector.reciprocal(rc32[:, :cols], sb[:, :cols])
                    rcb = nrm.tile([128, HA * N // 128], bf16, tag="rcb")
                    nc.vector.tensor_copy(rcb[:, :cols], rc32[:, :cols])
                    nc.sync.dma_start(
                        r_scr[0, :].rearrange("(p f) -> p f", p=128),
                        rcb[:, :cols])
                    nc.sync.dma_start(
                        rba[:, h0 * N:(h0 + nh) * N],
                        r_scr[:].to_broadcast([128, nh * N]))

                def norm_mul(h, engine=None):
                    ct, po = h // 2, 64 * (h % 2)
                    sl = atb[po:po + 64, ct, :]
                    eng = engine or nc.gpsimd
                    eng.tensor_tensor(
                        sl, sl, rba[po:po + 64, h * N:(h + 1) * N], ALU.mult)

                # ---- emission schedule --------------------------------
                EB_DEPTH = 4
                for t in range(EB_DEPTH):
                    eb_load(0, t)
                v_proj(0, range(NT))
                kq_ct(0)

                # fillers per pair: kq for pair+1, rolling EB prefetch,
                # v_proj block1 during pairs 2-3, norm batches late.
                for pr in range(NP):
                    fill = {j: [] for j in range(NT)}
                    # rolling EB prefetch, EB_DEPTH tiles ahead
                    for j in range(NT):
                        t = pr * NT + j + EB_DEPTH
                        if t < NP * NT:
                            fill[j].append(
                                lambda a=t // NT, b=t % NT: eb_load(a, b))
                    nxt = pr + 1
                    if nxt < NP:
                        fill[1].append(lambda n=nxt: kq_sub("k", n, 0))
                        fill[2].append(lambda n=nxt: kq_sub("k", n, 1))
                        fill[4].append(lambda n=nxt: kq_sub("q", n, 0))
                        fill[5].append(lambda n=nxt: kq_sub("q", n, 1))
                    if pr == 2:
                        fill[6].append(lambda: v_proj(1, range(0, 4)))
                    if pr == 3:
                        fill[6].append(lambda: v_proj(1, range(4, NT)))
                    if pr == 4:
                        # heads 0..7 sums are complete after pair 3 evac
                        fill[0].append(lambda: norm_batch(0))
                        for h in range(4):
                            fill[6].append(lambda hh=h: norm_mul(hh))
                    if pr == 5:
                        fill[0].append(lambda: norm_batch(1))
                        for h in range(4, 8):
                            fill[3].append(lambda hh=h: norm_mul(hh))
                        for h in range(8, 10):
                            fill[6].append(lambda hh=h: norm_mul(hh))
                    attn_pair(pr, fill)

                norm_batch(2)
                norm_mul(10, nc.vector)
                norm_mul(11, nc.vector)

            # ---- output projection ------------------------------------
            with tc.tile_pool(name="ops", bufs=4, space="PSUM") as pC, \
                 tc.tile_pool(name="otb", bufs=3) as otp:
                for cot in range(CT):
                    ps = pC.tile([128, N], f32, tag="o")
                    for nb in range(2):
                        for ci in range(CT):
                            nc.tensor.matmul(
                                ps[:, nb * F:(nb + 1) * F],
                                lhsT=wpb[:, ci, cot * 128:(cot + 1) * 128],
                                rhs=atb[:, ci, nb * F:(nb + 1) * F],
                                start=(ci == 0),
                                stop=(ci == CT - 1),
                            )
                    ot = otp.tile([128, N], f32, tag="ot")
                    nc.scalar.activation(
                        ot, ps, AF.Identity, bias=bpb[:, cot:cot + 1])
                    nc.gpsimd.dma_start(
                        outT_d[cot * 128:(cot + 1) * 128, :], ot)

    nc.compile()
    return nc


def _get_nc():
    if "nc" not in _cache:
        _cache["nc"] = _build()
    return _cache["nc"]


def prep_in_maps(x, attn_bias, Wq, Wk, Wv, Wp, bp):
    """Host-side sharding + layout prep (transposes/casts/exp of bias)."""
    wqT = np.ascontiguousarray(Wq.T).astype(BF16)
    wkT = np.ascontiguousarray(Wk.T).astype(BF16)
    wvT = np.ascontiguousarray(Wv.T).astype(BF16)
    wpT = np.ascontiguousarray(Wp.T).astype(BF16)
    bpT = np.ascontiguousarray(bp.astype(np.float32).reshape(CT, 128).T)
    # exp(bias)^T packed per (pair, key-tile, query-half): see kernel docstr
    E = np.exp(attn_bias[0].astype(np.float32)).transpose(0, 2, 1)
    E = np.ascontiguousarray(E).reshape(H, NT, 128, 2, F)
    ebPk = np.empty((NP, NT, 2, 128, 2 * F), dtype=np.float32)
    for pr in range(NP):
        ebPk[pr, :, :, :, 0:F] = E[2 * pr].transpose(0, 2, 1, 3)
        ebPk[pr, :, :, :, F:2 * F] = E[2 * pr + 1].transpose(0, 2, 1, 3)
    ebPk = ebPk.astype(BF16)
    in_maps = []
    for b in range(B):
        in_maps.append({
            "xT": np.ascontiguousarray(x[b].T).astype(BF16),
            "wqT": wqT, "wkT": wkT, "wvT": wvT, "wpT": wpT,
            "bpT": bpT, "ebPk": ebPk,
        })
    return in_maps


def run(in_maps, trace=False, **kw):
    from concourse.bass_utils import run_bass_kernel_spmd

    nc = _get_nc()
    return run_bass_kernel_spmd(
        nc, in_maps, core_ids=list(range(B)), trace=trace, **kw
    )


def kernel(x, attn_bias, Wq, Wk, Wv, Wp, bp):
    res = run(prep_in_maps(x, attn_bias, Wq, Wk, Wv, Wp, bp))
    out = np.stack(
        [res.results[b]["outT"].T for b in range(B)]
    ).astype(np.float32)
    return out


# revision 10
# speedup vs baseline: 1.2068x; 1.0275x over previous
"""Multi-head attention (B=8, N=1024, C=768, H=12, D=64) on 8 TRN2 NeuronCores.

Strategy: pure data-parallel over batch (B == n_cores == 8), no collectives.
Each core computes full 12-head attention for one batch element in a fully
transposed layout (channels on SBUF partitions).

v2 design (vs. the nb-serial baseline):
  - Heads are processed in PAIRS (2i, 2i+1).  The even head's K/Q live on
    SBUF partitions 0:64, the odd head's on 64:128, so the two QK^T matmuls
    (contraction 64 each) run CONCURRENTLY in the PE array via row tiling
    (tile_position (0,0) / (64,0)) writing different PSUM banks.
  - S-pair tiles are [128, 1024] f32 (2 PSUM banks): cols 0:512 even head,
    512:1024 odd head, for one (key-tile j, query-half nb).  One FD=1024
    ACT exp per tile halves the per-instruction overhead.
  - The additive attn bias is applied as exp(S/8)*exp(bias): exp(bias) is
    precomputed on host, loaded bf16, multiplied on DVE at 2x rate in SBUF
    (the f32-PSUM add of the baseline ran at 1x and cost 131us).
  - PV keeps the ones-column trick (out rows 0:64 = unnormalized out^T,
    row 64 = softmax sum).  All four (head, nb) chains evacuate via one
    [65,512] DVE copy to SBUF, then gpsimd DMAs split rows 0:64 -> atb
    (partition-shifted for odd heads) and row 64 -> s_stage.
  - Normalization in 3 batches (heads 0-7, 8-9, 10-11) overlapped with
    attention; the last two heads' normalize-multiply runs on DVE to
    shorten the tail.  Whole norm DMA chain rides the sync queue (FIFO).
  - K/Q projection tiles for pair i+1 are interleaved into pair i's
    attention to keep the PE dense (HAM un-throttled).
"""

import os
import sys
import numpy as np

for _p in ("/opt/trn_rl_repo", "/root/.axon_site/_ro/trn_rl_repo"):
    if os.path.isdir(_p) and _p not in sys.path:
        sys.path.append(_p)

import ml_dtypes

BF16 = ml_dtypes.bfloat16

B, N, C = 8, 1024, 768
H, D = 12, 64
CT = C // 128         # 6 channel tiles
NT = N // 128         # 8 key tiles
F = 512
NP = H // 2           # 6 head pairs
HA = 8                # heads in normalization batch A (then 8-9, 10-11)

_cache = {}


def _build():
    import concourse.bass as bass
    import concourse.tile as tile
    from concourse import bacc, mybir

    f32 = mybir.dt.float32
    bf16 = mybir.dt.bfloat16
    AF = mybir.ActivationFunctionType
    ALU = mybir.AluOpType

    nc = bacc.Bacc("TRN2", target_bir_lowering=False)

    xT_d = nc.dram_tensor("xT", [C, N], bf16, kind="ExternalInput")
    wqT_d = nc.dram_tensor("wqT", [C, C], bf16, kind="ExternalInput")
    wkT_d = nc.dram_tensor("wkT", [C, C], bf16, kind="ExternalInput")
    wvT_d = nc.dram_tensor("wvT", [C, C], bf16, kind="ExternalInput")
    wpT_d = nc.dram_tensor("wpT", [C, C], bf16, kind="ExternalInput")
    bpT_d = nc.dram_tensor("bpT", [128, CT], f32, kind="ExternalInput")
    # exp(attn_bias) packed per (pair, key-tile j, query-half nb):
    # [...,0:512] = even head, [...,512:1024] = odd head
    eb_d = nc.dram_tensor("ebPk", [NP, NT, 2, 128, 2 * F], bf16,
                          kind="ExternalInput")
    outT_d = nc.dram_tensor("outT", [C, N], f32, kind="ExternalOutput")
    # softmax-sum scratch per normalization batch
    sA_scr = nc.dram_tensor("sA_scr", [HA * N], bf16)
    sB1_scr = nc.dram_tensor("sB1_scr", [2 * N], bf16)
    sB2_scr = nc.dram_tensor("sB2_scr", [2 * N], bf16)
    rA_scr = nc.dram_tensor("rA_scr", [1, HA * N], bf16)
    rB1_scr = nc.dram_tensor("rB1_scr", [1, 2 * N], bf16)
    rB2_scr = nc.dram_tensor("rB2_scr", [1, 2 * N], bf16)

    with tile.TileContext(nc) as tc:
        with tc.tile_pool(name="persist", bufs=1) as pers:
            xTb = pers.tile([128, CT, N], bf16, tag="xT")
            wqb = pers.tile([128, CT, C], bf16, tag="wq")
            wkb = pers.tile([128, CT, C], bf16, tag="wk")
            wvb = pers.tile([128, CT, C], bf16, tag="wv")
            wpb = pers.tile([128, CT, C], bf16, tag="wp")
            bpb = pers.tile([128, CT], f32, tag="bp")
            # row 64 collects softmax sums (same partition as pv row 64)
            s_stage = pers.tile([65, H * N], bf16, tag="s_stage")
            rba = pers.tile([128, H * N], bf16, tag="rba")
            qtb = pers.tile([128, CT, N], bf16, tag="qt")
            ktb = pers.tile([128, CT, N], bf16, tag="kt")
            vb = pers.tile([128, NT, H, D + 1], bf16, tag="v")
            atb = pers.tile([128, CT, N], bf16, tag="at")

            # Startup loads ordered so the first compute (kq_ct(0) and the
            # heads-0:4 V projection) can begin as soon as possible: small
            # leading slices of wk/wq/wv first, bulk later, wp deferred.
            for q0 in range(0, N, 256):
                nc.sync.dma_start(
                    xTb[:, :, q0:q0 + 256],
                    xT_d[:, q0:q0 + 256].rearrange(
                        "(ci p) n -> p ci n", p=128))
            nc.scalar.dma_start(
                wvb[:, :, 0:256],
                wvT_d[:, 0:256].rearrange("(ci p) o -> p ci o", p=128))
            nc.scalar.dma_start(
                wqb[:, :, 0:128],
                wqT_d[:, 0:128].rearrange("(ci p) o -> p ci o", p=128))
            nc.sync.dma_start(
                wkb[:, :, 0:128],
                wkT_d[:, 0:128].rearrange("(ci p) o -> p ci o", p=128))
            nc.sync.dma_start(
                wkb[:, :, 128:C],
                wkT_d[:, 128:C].rearrange("(ci p) o -> p ci o", p=128))
            nc.scalar.dma_start(
                wvb[:, :, 256:512],
                wvT_d[:, 256:512].rearrange("(ci p) o -> p ci o", p=128))
            nc.scalar.dma_start(
                wqb[:, :, 128:C],
                wqT_d[:, 128:C].rearrange("(ci p) o -> p ci o", p=128))
            nc.scalar.dma_start(
                wvb[:, :, 512:C],
                wvT_d[:, 512:C].rearrange("(ci p) o -> p ci o", p=128))
            nc.scalar.dma_start(bpb, bpT_d[:])

            nc.vector.memset(vb[:, :, :, D:D + 1], 1.0)

            with tc.tile_pool(name="ups", bufs=2, space="PSUM") as pU, \
                 tc.tile_pool(name="pvps", bufs=4, space="PSUM") as pPV, \
                 tc.tile_pool(name="ebb", bufs=4) as ebp, \
                 tc.tile_pool(name="vstagb", bufs=4) as vstagp, \
                 tc.tile_pool(name="nrmb", bufs=1) as nrm, \
                 tc.tile_pool(name="ptb", bufs=6) as ptp:

                eb_tiles = {}

                def eb_load(pr, j):
                    ebt = ebp.tile([128, 2, 2 * F], bf16, tag="eb")
                    nc.sync.dma_start(
                        ebt, eb_d[pr, j].rearrange("nb p q -> p nb q"))
                    eb_tiles[(pr, j)] = ebt

                def v_proj(h0, nh, nt):
                    """V projection for heads [h0, h0+nh) at key-tile nt."""
                    f0, fw = h0 * D, nh * D
                    ps = pU.tile([128, 2 * F], f32, tag="ps")
                    for ci in range(CT):
                        nc.tensor.matmul(
                            ps[:, :fw],
                            lhsT=xTb[:, ci, nt * 128:(nt + 1) * 128],
                            rhs=wvb[:, ci, f0:f0 + fw],
                            start=(ci == 0),
                            stop=(ci == CT - 1),
                        )
                    nc.vector.tensor_copy(
                        vb[:, nt, h0:h0 + nh, 0:D],
                        ps[:, :fw].rearrange("p (h d) -> p h d", d=D),
                    )

                def kq_sub(which, cot, nb):
                    wb, dst = (wkb, ktb) if which == "k" else (wqb, qtb)
                    ps = pU.tile([128, 2 * F], f32, tag="ps")
                    for ci in range(CT):
                        nc.tensor.matmul(
                            ps[:, :F],
                            lhsT=wb[:, ci, cot * 128:(cot + 1) * 128],
                            rhs=xTb[:, ci, nb * F:(nb + 1) * F],
                            start=(ci == 0),
                            stop=(ci == CT - 1),
                        )
                    nc.vector.tensor_copy(
                        dst[:, cot, nb * F:(nb + 1) * F], ps[:, :F])

                def kq_ct(cot):
                    for which in ("k", "q"):
                        for nb in range(2):
                            kq_sub(which, cot, nb)

                def qk_pair(pr, j, nb):
                    """Row-tiled QK^T for both heads of the pair: even head
                    on PE rows 0:64, odd head on rows 64:128, concurrent."""
                    ksl = slice(j * 128, (j + 1) * 128)
                    qsl = slice(nb * F, (nb + 1) * F)
                    sp = pU.tile([128, 2 * F], f32, tag="ps",
                                 name=f"sp_{pr}_{j}_{nb}")
                    nc.tensor.matmul(
                        sp[:, 0:F],
                        lhsT=ktb[0:64, pr, ksl],
                        rhs=qtb[0:64, pr, qsl],
                        start=True, stop=True,
                    )
                    nc.tensor.matmul(
                        sp[:, F:2 * F],
                        lhsT=ktb[64:128, pr, ksl],
                        rhs=qtb[64:128, pr, qsl],
                        start=True, stop=True,
                    )
                    return sp

                def attn_pair(pr, fillers):
                    """Attention for heads (2*pr, 2*pr+1).

                    Emission order per j: exp/mul for j, then QK for j+1
                    (so the PE never head-of-line blocks on the softmax
                    chain), then the four PV matmuls for j, then fillers.
                    """
                    pvs = [[pPV.tile([D + 1, F], f32, tag="pv",
                                     name=f"pv_{pr}_{hl}_{nb}")
                            for nb in range(2)] for hl in range(2)]
                    sps = [qk_pair(pr, 0, nb) for nb in range(2)]
                    for j in range(NT):
                        ebt = eb_tiles.pop((pr, j))
                        pts = []
                        for nb in range(2):
                            pt = ptp.tile([128, 2 * F], bf16, tag="pt",
                                          name=f"pt_{pr}_{j}_{nb}")
                            nc.scalar.activation(
                                pt, sps[nb], AF.Exp, scale=0.125)
                            nc.vector.tensor_tensor(
                                pt, pt, ebt[:, nb, :], ALU.mult)
                            pts.append(pt)
                        if j + 1 < NT:
                            sps = [qk_pair(pr, j + 1, nb) for nb in range(2)]
                        for nb in range(2):
                            for hl in range(2):
                                nc.tensor.matmul(
                                    pvs[hl][nb],
                                    lhsT=vb[:, j, 2 * pr + hl, :],
                                    rhs=pts[nb][:, hl * F:(hl + 1) * F],
                                    start=(j == 0),
                                    stop=(j == NT - 1),
                                )
                        for fn in fillers.get(j, ()):
                            fn()
                    # evacuate the 4 chains
                    for hl in range(2):
                        h = 2 * pr + hl
                        for nb in range(2):
                            qsl = slice(nb * F, (nb + 1) * F)
                            vstag = vstagp.tile([D + 1, F], bf16, tag="vstag")
                            nc.vector.tensor_copy(vstag, pvs[hl][nb])
                            nc.gpsimd.dma_start(
                                atb[64 * hl:64 * hl + 64, pr, qsl],
                                vstag[0:D, :])
                            nc.gpsimd.dma_start(
                                s_stage[D:D + 1,
                                        h * N + nb * F:h * N + (nb + 1) * F],
                                vstag[D:D + 1, :])

                def norm_batch(batch):
                    """Batched reciprocal of softmax sums for a head range.

                    All DMAs ride the sync queue so the DRAM round trip
                    is FIFO-ordered.
                    """
                    h0, nh, s_scr, r_scr = [
                        (0, HA, sA_scr, rA_scr),
                        (HA, 2, sB1_scr, rB1_scr),
                        (HA + 2, 2, sB2_scr, rB2_scr),
                    ][batch]
                    cols = nh * N // 128
                    nc.sync.dma_start(
                        s_scr[:], s_stage[D:D + 1, h0 * N:(h0 + nh) * N])
                    sb = nrm.tile([128, HA * N // 128], bf16, tag="sb")
                    nc.sync.dma_start(
                        sb[:, :cols],
                        s_scr[:].rearrange("(p f) -> p f", p=128))
                    rc32 = nrm.tile([128, HA * N // 128], f32, tag="rc32")
                    nc.vector.reciprocal(rc32[:, :cols], sb[:, :cols])
                    rcb = nrm.tile([128, HA * N // 128], bf16, tag="rcb")
                    nc.vector.tensor_copy(rcb[:, :cols], rc32[:, :cols])
                    nc.sync.dma_start(
                        r_scr[0, :].rearrange("(p f) -> p f", p=128),
                        rcb[:, :cols])
                    nc.sync.dma_start(
                        rba[:, h0 * N:(h0 + nh) * N],
                        r_scr[:].to_broadcast([128, nh * N]))

                def norm_mul(h, engine=None):
                    ct, po = h // 2, 64 * (h % 2)
                    sl = atb[po:po + 64, ct, :]
                    eng = engine or nc.gpsimd
                    eng.tensor_tensor(
                        sl, sl, rba[po:po + 64, h * N:(h + 1) * N], ALU.mult)

                # ---- emission schedule --------------------------------
                EB_DEPTH = 4
                for t in range(EB_DEPTH):
                    eb_load(0, t)
                kq_ct(0)
                nc.sync.dma_start(
                    wpb, wpT_d[:].rearrange("(ci p) o -> p ci o", p=128))
                v_proj(0, 4, 0)
                v_proj(0, 4, 1)

                # fillers per pair: kq projection for pair+1, V projection
                # slices for later pairs (heads 0:4 inside pair 0, 4:8 in
                # pair 1, 8:10 in pair 3, 10:12 in pair 4 - each finishing
                # just before the pair that consumes it), rolling EB
                # prefetch, norm batches late.  This back-loads PE filler
                # so the PE stays dense (HAM warm) through the last pairs.
                for pr in range(NP):
                    fill = {j: [] for j in range(NT)}
                    for j in range(NT):
                        t = pr * NT + j + EB_DEPTH
                        if t < NP * NT:
                            fill[j].append(
                                lambda a=t // NT, b=t % NT: eb_load(a, b))
                    nxt = pr + 1
                    if nxt < NP:
                        fill[1].append(lambda n=nxt: kq_sub("k", n, 0))
                        fill[2].append(lambda n=nxt: kq_sub("k", n, 1))
                        fill[4].append(lambda n=nxt: kq_sub("q", n, 0))
                        fill[5].append(lambda n=nxt: kq_sub("q", n, 1))
                    if pr == 0:
                        for nt in range(2, NT):
                            fill[nt - 2].append(
                                lambda t=nt: v_proj(0, 4, t))
                    if pr == 1:
                        for nt in range(NT):
                            fill[nt].append(lambda t=nt: v_proj(4, 4, t))
                    if pr == 3:
                        for nt in range(NT):
                            fill[nt].append(lambda t=nt: v_proj(8, 2, t))
                    if pr == 4:
                        # heads 0..7 sums are complete after pair 3 evac
                        fill[0].append(lambda: norm_batch(0))
                        for nt in range(NT):
                            fill[nt].append(lambda t=nt: v_proj(10, 2, t))
                        for h in range(4):
                            fill[5 + h // 2].append(lambda hh=h: norm_mul(hh))
                    if pr == 5:
                        fill[0].append(lambda: norm_batch(1))
                        for h in range(4, 8):
                            fill[2 + h // 2].append(lambda hh=h: norm_mul(hh))
                        for h in range(8, 10):
                            fill[6].append(lambda hh=h: norm_mul(hh))
                    attn_pair(pr, fill)

                norm_batch(2)

            # ---- output projection ------------------------------------
            # [128, 512] granularity (one PSUM bank per tile, 8 in
            # flight).  Pass 1 (ci 0..4: heads 0..9, already normalized)
            # for the first 8 tiles is emitted BEFORE the heads-10/11
            # normalize-multiplies so the PE stays busy through the final
            # normalization round trip; each tile then finishes with the
            # ci=5 matmul, a small ACT bias-add, and a store on the (idle)
            # sync queue.  Emitting more than 8 pass-1 tiles would
            # deadlock the PE FIFO on pool-slot reuse.
            with tc.tile_pool(name="ops", bufs=8, space="PSUM") as pC, \
                 tc.tile_pool(name="otb", bufs=6) as otp:

                def oproj_acc(cot, nb, cis):
                    ps = pC.tile([128, F], f32, tag="o",
                                 name=f"po_{cot}_{nb}")
                    for i, ci in enumerate(cis):
                        nc.tensor.matmul(
                            ps,
                            lhsT=wpb[:, ci, cot * 128:(cot + 1) * 128],
                            rhs=atb[:, ci, nb * F:(nb + 1) * F],
                            start=(i == 0),
                            stop=False,
                        )
                    return ps

                def oproj_fin(ps, cot, nb):
                    nc.tensor.matmul(
                        ps,
                        lhsT=wpb[:, CT - 1, cot * 128:(cot + 1) * 128],
                        rhs=atb[:, CT - 1, nb * F:(nb + 1) * F],
                        start=False, stop=True,
                    )
                    ot = otp.tile([128, F], f32, tag="ot",
                                  name=f"ot_{cot}_{nb}")
                    nc.scalar.activation(
                        ot, ps, AF.Identity, bias=bpb[:, cot:cot + 1])
                    nc.sync.dma_start(
                        outT_d[cot * 128:(cot + 1) * 128,
                               nb * F:(nb + 1) * F], ot)

                pss = {}
                for cot in range(4):
                    for nb in range(2):
                        pss[(cot, nb)] = oproj_acc(cot, nb, range(CT - 1))
                norm_mul(10, nc.vector)
                norm_mul(11, nc.vector)
                for cot in range(4):
                    for nb in range(2):
                        oproj_fin(pss[(cot, nb)], cot, nb)
                for cot in range(4, CT):
                    for nb in range(2):
                        ps = oproj_acc(cot, nb, range(CT - 1))
                        oproj_fin(ps, cot, nb)

    nc.compile()
    return nc


def _get_nc():
    if "nc" not in _cache:
        _cache["nc"] = _build()
    return _cache["nc"]


def prep_in_maps(x, attn_bias, Wq, Wk, Wv, Wp, bp):
    """Host-side sharding + layout prep (transposes/casts/exp of bias)."""
    wqT = np.ascontiguousarray(Wq.T).astype(BF16)
    wkT = np.ascontiguousarray(Wk.T).astype(BF16)
    wvT = np.ascontiguousarray(Wv.T).astype(BF16)
    wpT = np.ascontiguousarray(Wp.T).astype(BF16)
    bpT = np.ascontiguousarray(bp.astype(np.float32).reshape(CT, 128).T)
    # exp(bias)^T packed per (pair, key-tile, query-half): see kernel docstr
    E = np.exp(attn_bias[0].astype(np.float32)).transpose(0, 2, 1)
    E = np.ascontiguousarray(E).reshape(H, NT, 128, 2, F)
    ebPk = np.empty((NP, NT, 2, 128, 2 * F), dtype=np.float32)
    for pr in range(NP):
        ebPk[pr, :, :, :, 0:F] = E[2 * pr].transpose(0, 2, 1, 3)
        ebPk[pr, :, :, :, F:2 * F] = E[2 * pr + 1].transpose(0, 2, 1, 3)
    ebPk = ebPk.astype(BF16)
    in_maps = []
    for b in range(B):
        in_maps.append({
            "xT": np.ascontiguousarray(x[b].T).astype(BF16),
            "wqT": wqT, "wkT": wkT, "wvT": wvT, "wpT": wpT,
            "bpT": bpT, "ebPk": ebPk,
        })
    return in_maps


def run(in_maps, trace=False, **kw):
    from concourse.bass_utils import run_bass_kernel_spmd

    nc = _get_nc()
    return run_bass_kernel_spmd(
        nc, in_maps, core_ids=list(range(B)), trace=trace, **kw
    )


def kernel(x, attn_bias, Wq, Wk, Wv, Wp, bp):
    res = run(prep_in_maps(x, attn_bias, Wq, Wk, Wv, Wp, bp))
    out = np.stack(
        [res.results[b]["outT"].T for b in range(B)]
    ).astype(np.float32)
    return out


# revision 13
# speedup vs baseline: 1.2623x; 1.0460x over previous
"""Multi-head attention (B=8, N=1024, C=768, H=12, D=64) on 8 TRN2 NeuronCores.

Strategy: pure data-parallel over batch (B == n_cores == 8), no collectives.
Each core computes full 12-head attention for one batch element in a fully
transposed layout (channels on SBUF partitions).

v2 design (vs. the nb-serial baseline):
  - Heads are processed in PAIRS (2i, 2i+1).  The even head's K/Q live on
    SBUF partitions 0:64, the odd head's on 64:128, so the two QK^T matmuls
    (contraction 64 each) run CONCURRENTLY in the PE array via row tiling
    (tile_position (0,0) / (64,0)) writing different PSUM banks.
  - S-pair tiles are [128, 1024] f32 (2 PSUM banks): cols 0:512 even head,
    512:1024 odd head, for one (key-tile j, query-half nb).  One FD=1024
    ACT exp per tile halves the per-instruction overhead.
  - The additive attn bias is applied as exp(S/8)*exp(bias): exp(bias) is
    precomputed on host, loaded bf16, multiplied on DVE at 2x rate in SBUF
    (the f32-PSUM add of the baseline ran at 1x and cost 131us).
  - PV keeps the ones-column trick (out rows 0:64 = unnormalized out^T,
    row 64 = softmax sum).  All four (head, nb) chains evacuate via one
    [65,512] DVE copy to SBUF, then gpsimd DMAs split rows 0:64 -> atb
    (partition-shifted for odd heads) and row 64 -> s_stage.
  - Normalization in 3 batches (heads 0-7, 8-9, 10-11) overlapped with
    attention; the last two heads' normalize-multiply runs on DVE to
    shorten the tail.  Whole norm DMA chain rides the sync queue (FIFO).
  - K/Q projection tiles for pair i+1 are interleaved into pair i's
    attention to keep the PE dense (HAM un-throttled).
"""

import os
import sys
import numpy as np

for _p in ("/opt/trn_rl_repo", "/root/.axon_site/_ro/trn_rl_repo"):
    if os.path.isdir(_p) and _p not in sys.path:
        sys.path.append(_p)

import ml_dtypes

BF16 = ml_dtypes.bfloat16

B, N, C = 8, 1024, 768
H, D = 12, 64
CT = C // 128         # 6 channel tiles
NT = N // 128         # 8 key tiles
F = 512
NP = H // 2           # 6 head pairs
HA = 8                # heads in normalization batch A (then 8-9, 10-11)

_cache = {}


def _build():
    import concourse.bass as bass
    import concourse.tile as tile
    from concourse import bacc, mybir

    f32 = mybir.dt.float32
    bf16 = mybir.dt.bfloat16
    AF = mybir.ActivationFunctionType
    ALU = mybir.AluOpType

    nc = bacc.Bacc("TRN2", target_bir_lowering=False)

    xT_d = nc.dram_tensor("xT", [C, N], bf16, kind="ExternalInput")
    wqT_d = nc.dram_tensor("wqT", [C, C], bf16, kind="ExternalInput")
    wkT_d = nc.dram_tensor("wkT", [C, C], bf16, kind="ExternalInput")
    wvT_d = nc.dram_tensor("wvT", [C, C], bf16, kind="ExternalInput")
    wpT_d = nc.dram_tensor("wpT", [C, C], bf16, kind="ExternalInput")
    bpT_d = nc.dram_tensor("bpT", [128, CT], f32, kind="ExternalInput")
    # exp(attn_bias) packed per (pair, key-tile j, query-half nb):
    # [...,0:512] = even head, [...,512:1024] = odd head
    eb_d = nc.dram_tensor("ebPk", [NP, NT, 2, 128, 2 * F], bf16,
                          kind="ExternalInput")
    outT_d = nc.dram_tensor("outT", [C, N], f32, kind="ExternalOutput")
    # softmax-sum scratch per normalization batch
    sA_scr = nc.dram_tensor("sA_scr", [HA * N], bf16)
    sB1_scr = nc.dram_tensor("sB1_scr", [2 * N], bf16)
    sB2_scr = nc.dram_tensor("sB2_scr", [2 * N], bf16)
    rA_scr = nc.dram_tensor("rA_scr", [1, HA * N], bf16)
    rB1_scr = nc.dram_tensor("rB1_scr", [1, 2 * N], bf16)
    rB2_scr = nc.dram_tensor("rB2_scr", [1, 2 * N], bf16)

    with tile.TileContext(nc) as tc:
        with tc.tile_pool(name="persist", bufs=1) as pers:
            xTb = pers.tile([128, CT, N], bf16, tag="xT")
            wqb = pers.tile([128, CT, C], bf16, tag="wq")
            wkb = pers.tile([128, CT, C], bf16, tag="wk")
            wvb = pers.tile([128, CT, C], bf16, tag="wv")
            wpb = pers.tile([128, CT, C], bf16, tag="wp")
            bpb = pers.tile([128, CT], f32, tag="bp")
            # row 64 collects softmax sums (same partition as pv row 64)
            s_stage = pers.tile([65, H * N], bf16, tag="s_stage")
            rba = pers.tile([128, H * N], bf16, tag="rba")
            qtb = pers.tile([128, CT, N], bf16, tag="qt")
            ktb = pers.tile([128, CT, N], bf16, tag="kt")
            vb = pers.tile([128, NT, H, D + 1], bf16, tag="v")
            atb = pers.tile([128, CT, N], bf16, tag="at")

            for q0 in range(0, N, 256):
                nc.sync.dma_start(
                    xTb[:, :, q0:q0 + 256],
                    xT_d[:, q0:q0 + 256].rearrange(
                        "(ci p) n -> p ci n", p=128))
            nc.scalar.dma_start(
                wvb[:, :, 0:512],
                wvT_d[:, 0:512].rearrange("(ci p) o -> p ci o", p=128))
            nc.scalar.dma_start(
                wvb[:, :, 512:C],
                wvT_d[:, 512:C].rearrange("(ci p) o -> p ci o", p=128))
            nc.sync.dma_start(
                wkb, wkT_d[:].rearrange("(ci p) o -> p ci o", p=128))
            nc.scalar.dma_start(
                wqb, wqT_d[:].rearrange("(ci p) o -> p ci o", p=128))
            nc.scalar.dma_start(bpb, bpT_d[:])

            nc.vector.memset(vb[:, :, :, D:D + 1], 1.0)

            with tc.tile_pool(name="ups", bufs=2, space="PSUM") as pU, \
                 tc.tile_pool(name="pvps", bufs=4, space="PSUM") as pPV, \
                 tc.tile_pool(name="ebb", bufs=4) as ebp, \
                 tc.tile_pool(name="vstagb", bufs=4) as vstagp, \
                 tc.tile_pool(name="nrmb", bufs=1) as nrm, \
                 tc.tile_pool(name="ptb", bufs=6) as ptp:

                eb_tiles = {}

                def eb_load(pr, j):
                    ebt = ebp.tile([128, 2, 2 * F], bf16, tag="eb")
                    nc.sync.dma_start(
                        ebt, eb_d[pr, j].rearrange("nb p q -> p nb q"))
                    eb_tiles[(pr, j)] = ebt

                def v_proj(h0, nh, nt):
                    """V projection for heads [h0, h0+nh) at key-tile nt."""
                    f0, fw = h0 * D, nh * D
                    ps = pU.tile([128, 2 * F], f32, tag="ps")
                    for ci in range(CT):
                        nc.tensor.matmul(
                            ps[:, :fw],
                            lhsT=xTb[:, ci, nt * 128:(nt + 1) * 128],
                            rhs=wvb[:, ci, f0:f0 + fw],
                            start=(ci == 0),
                            stop=(ci == CT - 1),
                        )
                    nc.vector.tensor_copy(
                        vb[:, nt, h0:h0 + nh, 0:D],
                        ps[:, :fw].rearrange("p (h d) -> p h d", d=D),
                    )

                def pe_warm():
                    """Redundant 6-matmul group (recomputes k-projection
                    tile 0 into a dead PSUM tile, never read).  Emitted in
                    filler-starved stretches so the PE's activity monitor
                    does not re-throttle the clock (K=4/8) on micro-idle."""
                    ps = pU.tile([128, 2 * F], f32, tag="ps", name="warm")
                    for ci in range(CT):
                        nc.tensor.matmul(
                            ps[:, :F],
                            lhsT=wkb[:, ci, 0:128],
                            rhs=xTb[:, ci, 0:F],
                            start=(ci == 0),
                            stop=(ci == CT - 1),
                        )

                def kq_sub(which, cot, nb):
                    wb, dst = (wkb, ktb) if which == "k" else (wqb, qtb)
                    ps = pU.tile([128, 2 * F], f32, tag="ps")
                    for ci in range(CT):
                        nc.tensor.matmul(
                            ps[:, :F],
                            lhsT=wb[:, ci, cot * 128:(cot + 1) * 128],
                            rhs=xTb[:, ci, nb * F:(nb + 1) * F],
                            start=(ci == 0),
                            stop=(ci == CT - 1),
                        )
                    nc.vector.tensor_copy(
                        dst[:, cot, nb * F:(nb + 1) * F], ps[:, :F])

                def kq_ct(cot):
                    for which in ("k", "q"):
                        for nb in range(2):
                            kq_sub(which, cot, nb)

                def qk_pair(pr, j, nb):
                    """Row-tiled QK^T for both heads of the pair: even head
                    on PE rows 0:64, odd head on rows 64:128, concurrent."""
                    ksl = slice(j * 128, (j + 1) * 128)
                    qsl = slice(nb * F, (nb + 1) * F)
                    sp = pU.tile([128, 2 * F], f32, tag="ps",
                                 name=f"sp_{pr}_{j}_{nb}")
                    nc.tensor.matmul(
                        sp[:, 0:F],
                        lhsT=ktb[0:64, pr, ksl],
                        rhs=qtb[0:64, pr, qsl],
                        start=True, stop=True,
                    )
                    nc.tensor.matmul(
                        sp[:, F:2 * F],
                        lhsT=ktb[64:128, pr, ksl],
                        rhs=qtb[64:128, pr, qsl],
                        start=True, stop=True,
                    )
                    return sp

                def attn_pair(pr, fillers):
                    """Attention for heads (2*pr, 2*pr+1).

                    Emission order per j: exp/mul for j, then QK for j+1
                    (so the PE never head-of-line blocks on the softmax
                    chain), then the four PV matmuls for j, then fillers.
                    """
                    pvs = [[pPV.tile([D + 1, F], f32, tag="pv",
                                     name=f"pv_{pr}_{hl}_{nb}")
                            for nb in range(2)] for hl in range(2)]
                    sps = [qk_pair(pr, 0, nb) for nb in range(2)]
                    for j in range(NT):
                        ebt = eb_tiles.pop((pr, j))
                        pts = []
                        for nb in range(2):
                            pt = ptp.tile([128, 2 * F], bf16, tag="pt",
                                          name=f"pt_{pr}_{j}_{nb}")
                            nc.scalar.activation(
                                pt, sps[nb], AF.Exp, scale=0.125)
                            nc.vector.tensor_tensor(
                                pt, pt, ebt[:, nb, :], ALU.mult)
                            pts.append(pt)
                        if j + 1 < NT:
                            sps = [qk_pair(pr, j + 1, nb) for nb in range(2)]
                        for nb in range(2):
                            for hl in range(2):
                                nc.tensor.matmul(
                                    pvs[hl][nb],
                                    lhsT=vb[:, j, 2 * pr + hl, :],
                                    rhs=pts[nb][:, hl * F:(hl + 1) * F],
                                    start=(j == 0),
                                    stop=(j == NT - 1),
                                )
                        for fn in fillers.get(j, ()):
                            fn()
                    # evacuate the 4 chains
                    for hl in range(2):
                        h = 2 * pr + hl
                        for nb in range(2):
                            qsl = slice(nb * F, (nb + 1) * F)
                            vstag = vstagp.tile([D + 1, F], bf16, tag="vstag")
                            nc.vector.tensor_copy(vstag, pvs[hl][nb])
                            nc.gpsimd.dma_start(
                                atb[64 * hl:64 * hl + 64, pr, qsl],
                                vstag[0:D, :])
                            nc.gpsimd.dma_start(
                                s_stage[D:D + 1,
                                        h * N + nb * F:h * N + (nb + 1) * F],
                                vstag[D:D + 1, :])

                def norm_batch(batch):
                    """Batched reciprocal of softmax sums for a head range.

                    All DMAs ride the sync queue so the DRAM round trip
                    is FIFO-ordered.
                    """
                    h0, nh, s_scr, r_scr = [
                        (0, HA, sA_scr, rA_scr),
                        (HA, 2, sB1_scr, rB1_scr),
                        (HA + 2, 2, sB2_scr, rB2_scr),
                    ][batch]
                    cols = nh * N // 128
                    nc.sync.dma_start(
                        s_scr[:], s_stage[D:D + 1, h0 * N:(h0 + nh) * N])
                    sb = nrm.tile([128, HA * N // 128], bf16, tag="sb")
                    nc.sync.dma_start(
                        sb[:, :cols],
                        s_scr[:].rearrange("(p f) -> p f", p=128))
                    rc32 = nrm.tile([128, HA * N // 128], f32, tag="rc32")
                    nc.vector.reciprocal(rc32[:, :cols], sb[:, :cols])
                    rcb = nrm.tile([128, HA * N // 128], bf16, tag="rcb")
                    nc.vector.tensor_copy(rcb[:, :cols], rc32[:, :cols])
                    nc.sync.dma_start(
                        r_scr[0, :].rearrange("(p f) -> p f", p=128),
                        rcb[:, :cols])
                    nc.sync.dma_start(
                        rba[:, h0 * N:(h0 + nh) * N],
                        r_scr[:].to_broadcast([128, nh * N]))

                def norm_mul(h, engine=None):
                    ct, po = h // 2, 64 * (h % 2)
                    sl = atb[po:po + 64, ct, :]
                    eng = engine or nc.gpsimd
                    eng.tensor_tensor(
                        sl, sl, rba[po:po + 64, h * N:(h + 1) * N], ALU.mult)

                # ---- emission schedule --------------------------------
                EB_DEPTH = 4
                for t in range(EB_DEPTH):
                    eb_load(0, t)
                for nt in range(NT):
                    v_proj(0, 8, nt)
                nc.sync.dma_start(
                    wpb, wpT_d[:].rearrange("(ci p) o -> p ci o", p=128))
                kq_ct(0)

                # fillers per pair: kq projection for pair+1, rolling EB
                # prefetch, V projection for heads 8:12 during pairs 2-3,
                # norm batches late, PE warm-keepers in the filler-starved
                # pairs 4-5.
                for pr in range(NP):
                    fill = {j: [] for j in range(NT)}
                    for j in range(NT):
                        t = pr * NT + j + EB_DEPTH
                        if t < NP * NT:
                            fill[j].append(
                                lambda a=t // NT, b=t % NT: eb_load(a, b))
                    nxt = pr + 1
                    if nxt < NP:
                        fill[1].append(lambda n=nxt: kq_sub("k", n, 0))
                        fill[2].append(lambda n=nxt: kq_sub("k", n, 1))
                        fill[4].append(lambda n=nxt: kq_sub("q", n, 0))
                        fill[5].append(lambda n=nxt: kq_sub("q", n, 1))
                    if pr == 2:
                        for nt in range(4):
                            fill[6].append(lambda t=nt: v_proj(8, 4, t))
                    if pr == 3:
                        for nt in range(4, NT):
                            fill[6].append(lambda t=nt: v_proj(8, 4, t))
                    if pr == 4:
                        # heads 0..7 sums are complete after pair 3 evac
                        fill[0].append(lambda: norm_batch(0))
                        for h in range(4):
                            fill[6].append(lambda hh=h: norm_mul(hh))
                    if pr == 5:
                        fill[0].append(lambda: norm_batch(1))
                        for h in range(4, 8):
                            fill[3].append(lambda hh=h: norm_mul(hh))
                        for h in range(8, 10):
                            fill[5].append(lambda hh=h: norm_mul(hh))
                        for j in (1, 3, 5, 7):
                            fill[j].append(pe_warm)
                    attn_pair(pr, fill)

                norm_batch(2)

            # ---- output projection ------------------------------------
            # [128, 512] granularity (one PSUM bank per tile, 8 in
            # flight).  Pass 1 (ci 0..4: heads 0..9, already normalized)
            # for the first 8 tiles is emitted BEFORE the heads-10/11
            # normalize-multiplies so the PE stays busy through the final
            # normalization round trip; each tile then finishes with the
            # ci=5 matmul, a small ACT bias-add, and a store on the (idle)
            # sync queue.  Emitting more than 8 pass-1 tiles would
            # deadlock the PE FIFO on pool-slot reuse.
            with tc.tile_pool(name="ops", bufs=8, space="PSUM") as pC, \
                 tc.tile_pool(name="otb", bufs=6) as otp:

                def oproj_acc(cot, nb, cis):
                    ps = pC.tile([128, F], f32, tag="o",
                                 name=f"po_{cot}_{nb}")
                    for i, ci in enumerate(cis):
                        nc.tensor.matmul(
                            ps,
                            lhsT=wpb[:, ci, cot * 128:(cot + 1) * 128],
                            rhs=atb[:, ci, nb * F:(nb + 1) * F],
                            start=(i == 0),
                            stop=False,
                        )
                    return ps

                def oproj_fin(ps, cot, nb):
                    nc.tensor.matmul(
                        ps,
                        lhsT=wpb[:, CT - 1, cot * 128:(cot + 1) * 128],
                        rhs=atb[:, CT - 1, nb * F:(nb + 1) * F],
                        start=False, stop=True,
                    )
                    ot = otp.tile([128, F], f32, tag="ot",
                                  name=f"ot_{cot}_{nb}")
                    nc.scalar.activation(
                        ot, ps, AF.Identity, bias=bpb[:, cot:cot + 1])
                    nc.sync.dma_start(
                        outT_d[cot * 128:(cot + 1) * 128,
                               nb * F:(nb + 1) * F], ot)

                pss = {}
                for cot in range(4):
                    for nb in range(2):
                        pss[(cot, nb)] = oproj_acc(cot, nb, range(CT - 1))
                norm_mul(10, nc.vector)
                norm_mul(11, nc.vector)
                for cot in range(4):
                    for nb in range(2):
                        oproj_fin(pss[(cot, nb)], cot, nb)
                for cot in range(4, CT):
                    for nb in range(2):
                        ps = oproj_acc(cot, nb, range(CT - 1))
                        oproj_fin(ps, cot, nb)

    nc.compile()
    return nc


def _get_nc():
    if "nc" not in _cache:
        _cache["nc"] = _build()
    return _cache["nc"]


def prep_in_maps(x, attn_bias, Wq, Wk, Wv, Wp, bp):
    """Host-side sharding + layout prep (transposes/casts/exp of bias)."""
    wqT = np.ascontiguousarray(Wq.T).astype(BF16)
    wkT = np.ascontiguousarray(Wk.T).astype(BF16)
    wvT = np.ascontiguousarray(Wv.T).astype(BF16)
    wpT = np.ascontiguousarray(Wp.T).astype(BF16)
    bpT = np.ascontiguousarray(bp.astype(np.float32).reshape(CT, 128).T)
    # exp(bias)^T packed per (pair, key-tile, query-half): see kernel docstr
    E = np.exp(attn_bias[0].astype(np.float32)).transpose(0, 2, 1)
    E = np.ascontiguousarray(E).reshape(H, NT, 128, 2, F)
    ebPk = np.empty((NP, NT, 2, 128, 2 * F), dtype=np.float32)
    for pr in range(NP):
        ebPk[pr, :, :, :, 0:F] = E[2 * pr].transpose(0, 2, 1, 3)
        ebPk[pr, :, :, :, F:2 * F] = E[2 * pr + 1].transpose(0, 2, 1, 3)
    ebPk = ebPk.astype(BF16)
    in_maps = []
    for b in range(B):
        in_maps.append({
            "xT": np.ascontiguousarray(x[b].T).astype(BF16),
            "wqT": wqT, "wkT": wkT, "wvT": wvT, "wpT": wpT,
            "bpT": bpT, "ebPk": ebPk,
        })
    return in_maps


def run(in_maps, trace=False, **kw):
    from concourse.bass_utils import run_bass_kernel_spmd

    nc = _get_nc()
    return run_bass_kernel_spmd(
        nc, in_maps, core_ids=list(range(B)), trace=trace, **kw
    )


def kernel(x, attn_bias, Wq, Wk, Wv, Wp, bp):
    res = run(prep_in_maps(x, attn_bias, Wq, Wk, Wv, Wp, bp))
    out = np.stack(
        [res.results[b]["outT"].T for b in range(B)]
    ).astype(np.float32)
    return out


# revision 18
# speedup vs baseline: 1.3374x; 1.0595x over previous
"""Multi-head attention (B=8, N=1024, C=768, H=12, D=64) on 8 TRN2 NeuronCores.

Strategy: pure data-parallel over batch (B == n_cores == 8), no collectives.
Each core computes full 12-head attention for one batch element in a fully
transposed layout (channels on SBUF partitions).

v2 design (vs. the nb-serial baseline):
  - Heads are processed in PAIRS (2i, 2i+1).  The even head's K/Q live on
    SBUF partitions 0:64, the odd head's on 64:128, so the two QK^T matmuls
    (contraction 64 each) run CONCURRENTLY in the PE array via row tiling
    (tile_position (0,0) / (64,0)) writing different PSUM banks.
  - S-pair tiles are [128, 1024] f32 (2 PSUM banks): cols 0:512 even head,
    512:1024 odd head, for one (key-tile j, query-half nb).  One FD=1024
    ACT exp per tile halves the per-instruction overhead.
  - The additive attn bias is applied as exp(S/8)*exp(bias): exp(bias) is
    precomputed on host, loaded bf16, multiplied on DVE at 2x rate in SBUF
    (the f32-PSUM add of the baseline ran at 1x and cost 131us).
  - PV keeps the ones-column trick (out rows 0:64 = unnormalized out^T,
    row 64 = softmax sum).  All four (head, nb) chains evacuate via one
    [65,512] DVE copy to SBUF, then gpsimd DMAs split rows 0:64 -> atb
    (partition-shifted for odd heads) and row 64 -> s_stage.
  - Normalization in 3 batches (heads 0-7, 8-9, 10-11) overlapped with
    attention; the last two heads' normalize-multiply runs on DVE to
    shorten the tail.  Whole norm DMA chain rides the sync queue (FIFO).
  - K/Q projection tiles for pair i+1 are interleaved into pair i's
    attention to keep the PE dense (HAM un-throttled).
"""

import os
import sys
import numpy as np

for _p in ("/opt/trn_rl_repo", "/root/.axon_site/_ro/trn_rl_repo"):
    if os.path.isdir(_p) and _p not in sys.path:
        sys.path.append(_p)

import ml_dtypes

BF16 = ml_dtypes.bfloat16

B, N, C = 8, 1024, 768
H, D = 12, 64
CT = C // 128         # 6 channel tiles
NT = N // 128         # 8 key tiles
F = 512
NP = H // 2           # 6 head pairs
HA = 8                # heads in normalization batch A (then 8-9, 10-11)

_cache = {}


def _build():
    import concourse.bass as bass
    import concourse.tile as tile
    from concourse import bacc, mybir

    f32 = mybir.dt.float32
    bf16 = mybir.dt.bfloat16
    AF = mybir.ActivationFunctionType
    ALU = mybir.AluOpType

    nc = bacc.Bacc("TRN2", target_bir_lowering=False)

    xT_d = nc.dram_tensor("xT", [C, N], bf16, kind="ExternalInput")
    wqT_d = nc.dram_tensor("wqT", [C, C], bf16, kind="ExternalInput")
    wkT_d = nc.dram_tensor("wkT", [C, C], bf16, kind="ExternalInput")
    wvT_d = nc.dram_tensor("wvT", [C, C], bf16, kind="ExternalInput")
    wpT_d = nc.dram_tensor("wpT", [C, C], bf16, kind="ExternalInput")
    bpT_d = nc.dram_tensor("bpT", [128, CT], f32, kind="ExternalInput")
    # exp(attn_bias) packed per (pair, key-tile j, query-half nb):
    # [...,0:512] = even head, [...,512:1024] = odd head
    eb_d = nc.dram_tensor("ebPk", [NP, NT, 2, 128, 2 * F], bf16,
                          kind="ExternalInput")
    outT_d = nc.dram_tensor("outT", [C, N], f32, kind="ExternalOutput")
    # softmax-sum scratch per normalization batch
    sA_scr = nc.dram_tensor("sA_scr", [HA * N], bf16)
    sB1_scr = nc.dram_tensor("sB1_scr", [2 * N], bf16)
    sB2_scr = nc.dram_tensor("sB2_scr", [2 * N], bf16)
    rA_scr = nc.dram_tensor("rA_scr", [1, HA * N], bf16)
    rB1_scr = nc.dram_tensor("rB1_scr", [1, 2 * N], bf16)
    rB2_scr = nc.dram_tensor("rB2_scr", [1, 2 * N], bf16)

    with tile.TileContext(nc) as tc:
        with tc.tile_pool(name="persist", bufs=1) as pers:
            xTb = pers.tile([128, CT, N], bf16, tag="xT")
            wqb = pers.tile([128, CT, C], bf16, tag="wq")
            wkb = pers.tile([128, CT, C], bf16, tag="wk")
            wvb = pers.tile([128, CT, C], bf16, tag="wv")
            wpb = pers.tile([128, CT, C], bf16, tag="wp")
            bpb = pers.tile([128, CT], f32, tag="bp")
            # row 64 collects softmax sums (same partition as pv row 64)
            rba = pers.tile([128, H * N], bf16, tag="rba")
            qtb = pers.tile([128, CT, N], bf16, tag="qt")
            ktb = pers.tile([128, CT, N], bf16, tag="kt")
            vb = pers.tile([128, NT, H, D + 1], bf16, tag="v")
            atb = pers.tile([128, CT, N], bf16, tag="at")

            for q0 in range(0, N, 256):
                nc.sync.dma_start(
                    xTb[:, :, q0:q0 + 256],
                    xT_d[:, q0:q0 + 256].rearrange(
                        "(ci p) n -> p ci n", p=128))
            nc.scalar.dma_start(
                wvb[:, :, 0:512],
                wvT_d[:, 0:512].rearrange("(ci p) o -> p ci o", p=128))
            nc.scalar.dma_start(
                wvb[:, :, 512:C],
                wvT_d[:, 512:C].rearrange("(ci p) o -> p ci o", p=128))
            nc.sync.dma_start(
                wkb, wkT_d[:].rearrange("(ci p) o -> p ci o", p=128))
            nc.scalar.dma_start(
                wqb, wqT_d[:].rearrange("(ci p) o -> p ci o", p=128))
            nc.sync.dma_start(
                wpb, wpT_d[:].rearrange("(ci p) o -> p ci o", p=128))
            nc.scalar.dma_start(bpb, bpT_d[:])

            nc.vector.memset(vb[:, :, :, D:D + 1], 1.0)

            with tc.tile_pool(name="ups", bufs=2, space="PSUM") as pU, \
                 tc.tile_pool(name="pvps", bufs=4, space="PSUM") as pPV, \
                 tc.tile_pool(name="ebb", bufs=4) as ebp, \
                 tc.tile_pool(name="vstagb", bufs=4) as vstagp, \
                 tc.tile_pool(name="nrmb", bufs=1) as nrm, \
                 tc.tile_pool(name="ptb", bufs=6) as ptp:

                eb_tiles = {}

                def eb_load(pr, j):
                    ebt = ebp.tile([128, 2, 2 * F], bf16, tag="eb")
                    nc.sync.dma_start(
                        ebt, eb_d[pr, j].rearrange("nb p q -> p nb q"))
                    eb_tiles[(pr, j)] = ebt

                def v_proj(h0, nh, nt):
                    """V projection for heads [h0, h0+nh) at key-tile nt."""
                    f0, fw = h0 * D, nh * D
                    ps = pU.tile([128, 2 * F], f32, tag="ps")
                    for ci in range(CT):
                        nc.tensor.matmul(
                            ps[:, :fw],
                            lhsT=xTb[:, ci, nt * 128:(nt + 1) * 128],
                            rhs=wvb[:, ci, f0:f0 + fw],
                            start=(ci == 0),
                            stop=(ci == CT - 1),
                        )
                    nc.vector.tensor_copy(
                        vb[:, nt, h0:h0 + nh, 0:D],
                        ps[:, :fw].rearrange("p (h d) -> p h d", d=D),
                    )

                def pe_warm():
                    """Redundant 6-matmul group (recomputes k-projection
                    tile 0 into a dead PSUM tile, never read).  Emitted in
                    filler-starved stretches so the PE's activity monitor
                    does not re-throttle the clock (K=4/8) on micro-idle."""
                    ps = pU.tile([128, 2 * F], f32, tag="ps", name="warm")
                    for ci in range(CT):
                        nc.tensor.matmul(
                            ps[:, :F],
                            lhsT=wkb[:, ci, 0:128],
                            rhs=xTb[:, ci, 0:F],
                            start=(ci == 0),
                            stop=(ci == CT - 1),
                        )

                def kq_sub(which, cot, nb):
                    wb, dst = (wkb, ktb) if which == "k" else (wqb, qtb)
                    ps = pU.tile([128, 2 * F], f32, tag="ps")
                    for ci in range(CT):
                        nc.tensor.matmul(
                            ps[:, :F],
                            lhsT=wb[:, ci, cot * 128:(cot + 1) * 128],
                            rhs=xTb[:, ci, nb * F:(nb + 1) * F],
                            start=(ci == 0),
                            stop=(ci == CT - 1),
                        )
                    nc.vector.tensor_copy(
                        dst[:, cot, nb * F:(nb + 1) * F], ps[:, :F])

                def kq_ct(cot):
                    for which in ("k", "q"):
                        for nb in range(2):
                            kq_sub(which, cot, nb)

                def qk_pair(pr, j, nb):
                    """Row-tiled QK^T for both heads of the pair: even head
                    on PE rows 0:64, odd head on rows 64:128, concurrent."""
                    ksl = slice(j * 128, (j + 1) * 128)
                    qsl = slice(nb * F, (nb + 1) * F)
                    sp = pU.tile([128, 2 * F], f32, tag="ps",
                                 name=f"sp_{pr}_{j}_{nb}")
                    nc.tensor.matmul(
                        sp[:, 0:F],
                        lhsT=ktb[0:64, pr, ksl],
                        rhs=qtb[0:64, pr, qsl],
                        start=True, stop=True,
                    )
                    nc.tensor.matmul(
                        sp[:, F:2 * F],
                        lhsT=ktb[64:128, pr, ksl],
                        rhs=qtb[64:128, pr, qsl],
                        start=True, stop=True,
                    )
                    return sp

                def attn_pair(pr, fillers):
                    """Attention for heads (2*pr, 2*pr+1).

                    Emission order per j: exp/mul for j, then QK for j+1
                    (so the PE never head-of-line blocks on the softmax
                    chain), then the four PV matmuls for j, then fillers.
                    """
                    pvs = [[pPV.tile([D + 1, F], f32, tag="pv",
                                     name=f"pv_{pr}_{hl}_{nb}")
                            for nb in range(2)] for hl in range(2)]
                    sps = [qk_pair(pr, 0, nb) for nb in range(2)]
                    for j in range(NT):
                        ebt = eb_tiles.pop((pr, j))
                        pts = []
                        for nb in range(2):
                            pt = ptp.tile([128, 2 * F], bf16, tag="pt",
                                          name=f"pt_{pr}_{j}_{nb}")
                            nc.scalar.activation(
                                pt, sps[nb], AF.Exp, scale=0.125)
                            nc.vector.tensor_tensor(
                                pt, pt, ebt[:, nb, :], ALU.mult)
                            pts.append(pt)
                        if j + 1 < NT:
                            sps = [qk_pair(pr, j + 1, nb) for nb in range(2)]
                        for nb in range(2):
                            for hl in range(2):
                                nc.tensor.matmul(
                                    pvs[hl][nb],
                                    lhsT=vb[:, j, 2 * pr + hl, :],
                                    rhs=pts[nb][:, hl * F:(hl + 1) * F],
                                    start=(j == 0),
                                    stop=(j == NT - 1),
                                )
                        for fn in fillers.get(j, ()):
                            fn()
                    # evacuate the 4 chains; softmax-sum row 64 goes
                    # straight to the DRAM scratch its norm batch reads
                    for hl in range(2):
                        h = 2 * pr + hl
                        s_scr, hh = (
                            (sA_scr, h) if h < HA
                            else (sB1_scr, h - HA) if h < HA + 2
                            else (sB2_scr, h - HA - 2))
                        for nb in range(2):
                            qsl = slice(nb * F, (nb + 1) * F)
                            vstag = vstagp.tile([D + 1, F], bf16, tag="vstag")
                            nc.vector.tensor_copy(vstag, pvs[hl][nb])
                            nc.gpsimd.dma_start(
                                atb[64 * hl:64 * hl + 64, pr, qsl],
                                vstag[0:D, :])
                            nc.gpsimd.dma_start(
                                s_scr[hh * N + nb * F:hh * N + (nb + 1) * F],
                                vstag[D:D + 1, :])

                def norm_batch(batch):
                    """Batched reciprocal of softmax sums for a head range.

                    The sync-queue DMAs are FIFO-ordered among themselves;
                    the s_scr reload depends on the pair-evacuation DMAs
                    (gpsimd queue) via the DRAM tensor dependency.
                    """
                    h0, nh, s_scr, r_scr = [
                        (0, HA, sA_scr, rA_scr),
                        (HA, 2, sB1_scr, rB1_scr),
                        (HA + 2, 2, sB2_scr, rB2_scr),
                    ][batch]
                    cols = nh * N // 128
                    sb = nrm.tile([128, HA * N // 128], bf16, tag="sb")
                    nc.sync.dma_start(
                        sb[:, :cols],
                        s_scr[:].rearrange("(p f) -> p f", p=128))
                    rc32 = nrm.tile([128, HA * N // 128], f32, tag="rc32")
                    nc.vector.reciprocal(rc32[:, :cols], sb[:, :cols])
                    rcb = nrm.tile([128, HA * N // 128], bf16, tag="rcb")
                    nc.vector.tensor_copy(rcb[:, :cols], rc32[:, :cols])
                    nc.sync.dma_start(
                        r_scr[0, :].rearrange("(p f) -> p f", p=128),
                        rcb[:, :cols])
                    nc.sync.dma_start(
                        rba[:, h0 * N:(h0 + nh) * N],
                        r_scr[:].to_broadcast([128, nh * N]))

                def norm_mul(h, engine=None):
                    ct, po = h // 2, 64 * (h % 2)
                    sl = atb[po:po + 64, ct, :]
                    eng = engine or nc.gpsimd
                    eng.tensor_tensor(
                        sl, sl, rba[po:po + 64, h * N:(h + 1) * N], ALU.mult)

                # ---- emission schedule --------------------------------
                EB_DEPTH = 4
                for t in range(EB_DEPTH):
                    eb_load(0, t)
                for nt in range(NT):
                    v_proj(0, 8, nt)
                kq_ct(0)

                # fillers per pair: kq projection for pair+1, rolling EB
                # prefetch, V projection for heads 8:12 during pairs 2-3,
                # norm batches late, PE warm-keepers in the filler-starved
                # pairs 4-5.
                for pr in range(NP):
                    fill = {j: [] for j in range(NT)}
                    for j in range(NT):
                        t = pr * NT + j + EB_DEPTH
                        if t < NP * NT:
                            fill[j].append(
                                lambda a=t // NT, b=t % NT: eb_load(a, b))
                    nxt = pr + 1
                    if nxt < NP:
                        fill[1].append(lambda n=nxt: kq_sub("k", n, 0))
                        fill[2].append(lambda n=nxt: kq_sub("k", n, 1))
                        fill[4].append(lambda n=nxt: kq_sub("q", n, 0))
                        fill[5].append(lambda n=nxt: kq_sub("q", n, 1))
                    if pr == 2:
                        for nt in range(4):
                            fill[6].append(lambda t=nt: v_proj(8, 4, t))
                    if pr == 3:
                        for nt in range(4, NT):
                            fill[6].append(lambda t=nt: v_proj(8, 4, t))
                    if pr == 4:
                        # heads 0..7 sums are complete after pair 3 evac
                        fill[0].append(lambda: norm_batch(0))
                        for h in range(4):
                            fill[6].append(lambda hh=h: norm_mul(hh))
                    if pr == 5:
                        fill[0].append(lambda: norm_batch(1))
                        for h in range(4, 8):
                            fill[3].append(lambda hh=h: norm_mul(hh))
                        for h in range(8, 10):
                            fill[5].append(lambda hh=h: norm_mul(hh))
                    attn_pair(pr, fill)

                norm_batch(2)

            # ---- output projection ------------------------------------
            # [128, 512] granularity (one PSUM bank per tile, 8 in
            # flight).  Pass 1 (ci 0..4: heads 0..9, already normalized)
            # for the first 8 tiles is emitted BEFORE the heads-10/11
            # normalize-multiplies so the PE stays busy through the final
            # normalization round trip; each tile then finishes with the
            # ci=5 matmul, a small ACT bias-add, and a store on the (idle)
            # sync queue.  Emitting more than 8 pass-1 tiles would
            # deadlock the PE FIFO on pool-slot reuse.
            with tc.tile_pool(name="ops", bufs=8, space="PSUM") as pC, \
                 tc.tile_pool(name="otb", bufs=6) as otp:

                def oproj_acc(cot, nb, cis):
                    ps = pC.tile([128, F], f32, tag="o",
                                 name=f"po_{cot}_{nb}")
                    for i, ci in enumerate(cis):
                        nc.tensor.matmul(
                            ps,
                            lhsT=wpb[:, ci, cot * 128:(cot + 1) * 128],
                            rhs=atb[:, ci, nb * F:(nb + 1) * F],
                            start=(i == 0),
                            stop=False,
                        )
                    return ps

                def oproj_fin(ps, cot, nb):
                    nc.tensor.matmul(
                        ps,
                        lhsT=wpb[:, CT - 1, cot * 128:(cot + 1) * 128],
                        rhs=atb[:, CT - 1, nb * F:(nb + 1) * F],
                        start=False, stop=True,
                    )
                    ot = otp.tile([128, F], f32, tag="ot",
                                  name=f"ot_{cot}_{nb}")
                    nc.scalar.activation(
                        ot, ps, AF.Identity, bias=bpb[:, cot:cot + 1])
                    nc.sync.dma_start(
                        outT_d[cot * 128:(cot + 1) * 128,
                               nb * F:(nb + 1) * F], ot)

                pss = {}
                for cot in range(4):
                    for nb in range(2):
                        pss[(cot, nb)] = oproj_acc(cot, nb, range(CT - 1))
                norm_mul(10, nc.vector)
                norm_mul(11, nc.vector)
                for cot in range(4):
                    for nb in range(2):
                        oproj_fin(pss[(cot, nb)], cot, nb)
                for cot in range(4, CT):
                    for nb in range(2):
                        ps = oproj_acc(cot, nb, range(CT - 1))
                        oproj_fin(ps, cot, nb)

    nc.compile()
    return nc


def _get_nc():
    if "nc" not in _cache:
        _cache["nc"] = _build()
    return _cache["nc"]


def prep_in_maps(x, attn_bias, Wq, Wk, Wv, Wp, bp):
    """Host-side sharding + layout prep (transposes/casts/exp of bias)."""
    wqT = np.ascontiguousarray(Wq.T).astype(BF16)
    wkT = np.ascontiguousarray(Wk.T).astype(BF16)
    wvT = np.ascontiguousarray(Wv.T).astype(BF16)
    wpT = np.ascontiguousarray(Wp.T).astype(BF16)
    bpT = np.ascontiguousarray(bp.astype(np.float32).reshape(CT, 128).T)
    # exp(bias)^T packed per (pair, key-tile, query-half): see kernel docstr
    E = np.exp(attn_bias[0].astype(np.float32)).transpose(0, 2, 1)
    E = np.ascontiguousarray(E).reshape(H, NT, 128, 2, F)
    ebPk = np.empty((NP, NT, 2, 128, 2 * F), dtype=np.float32)
    for pr in range(NP):
        ebPk[pr, :, :, :, 0:F] = E[2 * pr].transpose(0, 2, 1, 3)
        ebPk[pr, :, :, :, F:2 * F] = E[2 * pr + 1].transpose(0, 2, 1, 3)
    ebPk = ebPk.astype(BF16)
    in_maps = []
    for b in range(B):
        in_maps.append({
            "xT": np.ascontiguousarray(x[b].T).astype(BF16),
            "wqT": wqT, "wkT": wkT, "wvT": wvT, "wpT": wpT,
            "bpT": bpT, "ebPk": ebPk,
        })
    return in_maps


def run(in_maps, trace=False, **kw):
    from concourse.bass_utils import run_bass_kernel_spmd

    nc = _get_nc()
    return run_bass_kernel_spmd(
        nc, in_maps, core_ids=list(range(B)), trace=trace, **kw
    )


def kernel(x, attn_bias, Wq, Wk, Wv, Wp, bp):
    res = run(prep_in_maps(x, attn_bias, Wq, Wk, Wv, Wp, bp))
    out = np.stack(
        [res.results[b]["outT"].T for b in range(B)]
    ).astype(np.float32)
    return out
